# revision 29
# baseline (speedup 1.0000x reference)
"""Trainium2 Bass kernel for GCNN operator:
    h   = einsum('bnf,nfg->bng', x, kernel)   # per-node feature transform
    out = einsum('nm,bmg->bng', A, h) + bias  # dense adjacency aggregation

Sharding: node dim N row-sharded across 8 cores for the A@h matmul only.
Every core redundantly computes the FULL h on its DVE (x and kernel are
small), so there is NO collective — no cross-core barrier, no skew
sensitivity. The A-shard (pre-transposed, pre-tiled on host) streams from
HBM while the TensorEngine accumulates out^T = sum_m H_m^T @ A^T_m.

The kernel is HBM-bandwidth bound on the A stream, so A is stored in
three precisions, interleaved in SPANS of 16 j-blocks:
  per span: [4 x bf16 | 5 x fp8e4 | 7 x int8 @ scale 2^-5]
bf16/fp8 tiles feed the PE directly as the moving operand (the bf16/fp8
ISA path allows 1024-row matmuls, halving PE instruction count vs fp16;
the stationary h stays fp16). int8 j-blocks are converted to bf16 by the
Activation engine (hw-measured ~1.85us/j-block; GpSimd's software path
is 10x slower and is NOT used); int * 2^-5 is exact in bf16 so the
dequant scale rides in the conversion. The fine interleave is the load-
balancing trick: the PE walks j ascending, so conversions, h chunks,
DMA arrivals and PE consumption all advance uniformly — no engine ever
waits long on a buffer-recycle semaphore (a contiguous i8 region would
stall the converter on stage-slot reuse for ~100us).

Queues: SP carries x/bias + the bf16 tiles + stores; the GpSimd queue
carries the kq chunk stream (deadline-paced for the DVE's h chunks) +
the fp8 tiles; the Activation queue carries the int8 raws, self-paced
between conversions. A few warm-up matmuls on the x tile start the PE
p-state ramp early.

Emission goes through one global event list ordered by estimated time;
the estimates only shape ORDER (writer-before-reader, slot-ring WAR
pairing, and trigger/compute interleave per engine).

Self-contained: hardcodes shapes; only imports concourse + numpy.
"""

import numpy as np

B, N, F, G = 2, 16384, 16, 16
NCORES = 8
P = 128                    # SBUF partitions
C = B * G                  # 32 fused (batch, out-feature) columns
NT = 512                   # matmul moving free-dim (ISA AP element cap)
SPAN = 16                  # j-blocks per precision span
SP16 = 4                   # bf16 j-blocks per span
SP8 = 5                    # fp8 j-blocks per span
ITILE = (2, 2, 3)          # int8 tile sizes within a span
JJ = 8                     # j-blocks per H compute chunk
AT_BUFS = 4                # direct A-stream buffering depth
RAW_BUFS = 3               # int8 raw ring depth
STG_BUFS = 4               # converted-tile ring depth
KS_BUFS = 6                # kq chunk ring depth
ISCALE = 2.0 ** -5         # int8 quantization scale
N_PRIME = 8                # PE warm-up matmuls


def _span_plan(jn):
    """Per-j-block encoding plan. Returns (f16_js, f8_js, i8_js, spans)
    where spans is a list of per-span dicts with tile descriptors."""
    if jn >= SPAN:
        nspan = jn // SPAN
        sp16, sp8 = SP16, SP8
    else:
        nspan = 1
        sp16 = max((jn * 2 * SP16) // (2 * SPAN), 1)
        sp8 = max((jn * 2 * SP8) // (2 * SPAN), 1)
    f16_js, f8_js, i8_js, spans = [], [], [], []
    for s in range(nspan):
        base = s * (jn // nspan)
        width = jn // nspan
        n16 = sp16
        n8 = sp8
        ni = width - n16 - n8
        sp = {"f16": [], "f8": [], "i8": []}
        js16 = list(range(base, base + n16))
        js8 = list(range(base + n16, base + n16 + n8))
        jsi = list(range(base + n16 + n8, base + width))
        sp["f16"].append((len(f16_js), js16[0], n16))
        f16_js += js16
        sp["f8"].append((len(f8_js), js8[0], n8))
        f8_js += js8
        o = 0
        it = list(ITILE)
        while o < ni:
            sz = it.pop(0) if it else ni - o
            sz = min(sz, ni - o)
            sp["i8"].append((len(i8_js) + o, jsi[0] + o, sz))
            o += sz
        i8_js += jsi
        spans.append(sp)
    return f16_js, f8_js, i8_js, spans


def build_nc(n=N, ncores=NCORES, at_bufs=AT_BUFS):
    """Build the per-core Bass program (SPMD: same program on all cores)."""
    import concourse.bass as bass
    import concourse.mybir as mybir
    import concourse.tile as tile
    from concourse import bacc

    f32 = mybir.dt.float32
    f16 = mybir.dt.float16
    bf16 = mybir.dt.bfloat16
    f8 = mybir.dt.float8e4
    i8 = mybir.dt.int8

    nl = n // ncores           # local output rows per core
    jn = n // P                # contraction j-blocks over FULL n
    f16_js, f8_js, i8_js, spans = _span_plan(jn)
    n16, n8, ni = len(f16_js), len(f8_js), len(i8_js)
    ntc = min(NT, nl)          # matmul moving free-dim
    nt_n = max(nl // ntc, 1)   # acc tiles

    # H chunks of JJ j-blocks
    jj = min(JJ, jn)
    ch_sizes = [jj] * (jn // jj)
    if jn % jj:
        ch_sizes.append(jn % jj)
    nch = len(ch_sizes)
    ch_off = [0]
    for sz in ch_sizes:
        ch_off.append(ch_off[-1] + sz)
    m2ch = []
    for i, sz in enumerate(ch_sizes):
        for jl in range(sz):
            m2ch.append((i, jl))

    nc = bacc.Bacc("TRN2", target_bir_lowering=False, debug=False, num_devices=1)

    at16 = nc.dram_tensor("at16", [P, n16, nl], bf16, kind="ExternalInput")
    at8 = nc.dram_tensor("at8", [P, n8, nl], f8, kind="ExternalInput")
    ati = nc.dram_tensor("ati", [P, ni, nl], i8, kind="ExternalInput")
    xq = nc.dram_tensor("xq", [P, jn * B * F], f16, kind="ExternalInput")
    kq = nc.dram_tensor("kq", [P, jn * G * F], f16, kind="ExternalInput")
    bsT = nc.dram_tensor("bsT", [C, nl], f16, kind="ExternalInput")
    outs = nc.dram_tensor("outs", [C, nl], f32, kind="ExternalOutput")

    kq_r = kq.ap().rearrange("p (j g f) -> p j g f", g=G, f=F)

    # ---- timing estimates (µs), used only for emission ordering ----
    H_DT = 6.0                 # µs per DVE h chunk (1 bcast mult + 2 reduces)
    CONV_US = 1.9 * nl / 2048  # µs per converted j-block on ACT
    WIRE = 2.6                 # aggregate stream µs per MB
    mbytes = lambda nblk, bpe: P * nl * nblk * bpe / 1e6

    events = []

    def ev(t, fn):
        events.append((t, len(events), fn))

    with tile.TileContext(nc) as tc:
        with (
            tc.tile_pool(name="const", bufs=1) as const,
            tc.tile_pool(name="work", bufs=2) as work,
            tc.tile_pool(name="ksp", bufs=KS_BUFS) as ksp,
            tc.tile_pool(name="atp", bufs=at_bufs) as atp,
            tc.tile_pool(name="rawp", bufs=RAW_BUFS) as rawp,
            tc.tile_pool(name="stgp", bufs=STG_BUFS) as stgp,
            tc.tile_pool(name="pacc", bufs=1, space="PSUM") as pacc,
        ):
            xs = const.tile([P, jn, B, F], f16)
            biasT = const.tile([C, nl], f16)
            ks_tiles = [None] * nch
            hqs = [
                const.tile([P, ch_sizes[q], C], f16, tag=f"hq{q}",
                           name=f"hq{q}")
                for q in range(nch)
            ]
            outT = work.tile([C, nl], f32, tag="outT")
            acc = [
                pacc.tile([C, ntc], f32, tag=f"acc{t}", name=f"acc{t}")
                for t in range(nt_n)
            ]
            pw = min(512, jn * B * F)
            pprime = pacc.tile([C, pw], f32, tag="pprime")
            xs_flat = xs.rearrange("p a b c -> p (a b c)")

            # -- prologue on the sync queue: ks0 rides FIRST so the DVE's
            # first h chunk can start at ~7us, then xs, then bias --
            def load_ks0():
                kt = ksp.tile([P, jj, G, F], f16, tag="ks", name="ks0")
                nc.sync.dma_start(
                    out=kt[:, : ch_sizes[0], :, :], in_=kq_r[:, 0 : ch_off[1]]
                )
                ks_tiles[0] = kt
            ev(0.0, load_ks0)
            ev(0.005, lambda: nc.sync.dma_start(
                out=xs[:, :, :, :],
                in_=xq.ap().rearrange("p (j b f) -> p j b f", b=B, f=F),
            ))
            ev(0.01, lambda: nc.sync.dma_start(out=biasT[:, :], in_=bsT.ap()))

            # -- PE warm-up: p-state ramp starts once x lands --
            def prime():
                for _ in range(N_PRIME):
                    nc.tensor.matmul(
                        pprime[:, :], xs_flat[:, 0:C], xs_flat[:, 0:pw],
                        start=True, stop=True,
                    )
            ev(0.02, prime)

            # -- kq chunk ring on the gpsimd queue: a KS_BUFS-deep ring
            # paced just ahead of the DVE's h chunks (the slot WAR pairs
            # load q with the DVE's reads of chunk q-KS_BUFS) --
            for q in range(1, nch):
                def load_ks(q=q):
                    kt = ksp.tile([P, jj, G, F], f16, tag="ks",
                                  name=f"ks{q}")
                    nc.gpsimd.dma_start(
                        out=kt[:, : ch_sizes[q], :, :],
                        in_=kq_r[:, ch_off[q] : ch_off[q + 1]],
                    )
                    ks_tiles[q] = kt
                if q < 4:
                    t = 0.03 + 0.01 * q
                else:
                    t = 1.5 + H_DT * (q - 4)
                ev(t, load_ks)

            # -- h chunks: the otherwise-idle GpSimd computes batch b=1's
            # broadcast multiply while the DVE does b=0's mult plus both
            # reduces — the DVE chain drops from ~6.9us to ~5.7us/chunk
            # and stops being the span pacer --
            prod_b = [None] * nch
            for q in range(nch):
                sz = ch_sizes[q]
                def pmult(q=q, sz=sz):
                    pb = work.tile([P, jj, G, F], f16, tag="prodB",
                                   name=f"pb{q}")
                    with nc.allow_low_precision(reason="h mult in fp16"):
                        nc.gpsimd.tensor_tensor(
                            pb[:, :sz, :, :],
                            xs[
                                :, ch_off[q] : ch_off[q + 1], 1, None, :
                            ].to_broadcast([P, sz, G, F]),
                            ks_tiles[q][:, :sz, :, :],
                            mybir.AluOpType.mult,
                        )
                    prod_b[q] = pb
                def hstep(q=q, sz=sz):
                    with nc.allow_low_precision(
                        reason="h accum over F=16 in fp16"
                    ):
                        prod = work.tile([P, jj, G, F], f16, tag="prod")
                        nc.vector.tensor_tensor(
                            prod[:, :sz, :, :],
                            xs[
                                :, ch_off[q] : ch_off[q + 1], 0, None, :
                            ].to_broadcast([P, sz, G, F]),
                            ks_tiles[q][:, :sz, :, :],
                            mybir.AluOpType.mult,
                        )
                        nc.vector.tensor_reduce(
                            hqs[q][:, :, 0:G],
                            prod[:, :sz, :, :],
                            axis=mybir.AxisListType.X,
                            op=mybir.AluOpType.add,
                        )
                        nc.vector.tensor_reduce(
                            hqs[q][:, :, G : 2 * G],
                            prod_b[q][:, :sz, :, :],
                            axis=mybir.AxisListType.X,
                            op=mybir.AluOpType.add,
                        )
                ev(max(1.0 + H_DT * q - 4.6, 0.2 + 0.02 * q), pmult)
                ev(1.0 + H_DT * q, hstep)

            # -- per-tile emitters --
            at_tiles = {}
            raw_tiles = {}
            stg_tiles = {}

            def emit_direct(key, seg, so, sz, eng_i):
                def trig():
                    dt_t = bf16 if seg == 0 else f8
                    src = at16 if seg == 0 else at8
                    at_t = atp.tile([P, sz, nl], dt_t, tag="at_t",
                                    name="at_t")
                    eng = [nc.sync, nc.scalar, nc.gpsimd][eng_i]
                    eng.dma_start(
                        out=at_t[:, :, :], in_=src.ap()[:, so : so + sz, :]
                    )
                    at_tiles[key] = at_t
                return trig

            def emit_rawload(key, so, sz):
                def rawload():
                    rt = rawp.tile([P, max(ITILE), nl], i8, tag="rawt",
                                   name=f"raw{key}")
                    nc.scalar.dma_start(
                        out=rt[:, :sz, :], in_=ati.ap()[:, so : so + sz, :]
                    )
                    raw_tiles[key] = rt
                return rawload

            def emit_conv(key, sz):
                def conv():
                    rt = raw_tiles[key]
                    st = stgp.tile([P, max(ITILE), nl], bf16, tag="stgt",
                                   name=f"stg{key}")
                    with nc.allow_low_precision(
                        reason="int8*2^-5 dequant is exact in bf16"
                    ):
                        nc.scalar.activation(
                            st[:, :sz, :], rt[:, :sz, :],
                            mybir.ActivationFunctionType.Copy, scale=ISCALE,
                        )
                    stg_tiles[key] = st
                return conv

            # -- build the PE consumption sequence: span by span, j
            # ascending: f16 tile, f8 tile, then the span's i8 tiles --
            seq = []  # (kind, key, seg_off, abs_j0, sz)
            for sp in spans:
                for so, j0, sz in sp["f16"]:
                    seq.append(("d0", len(seq), so, j0, sz))
                for so, j0, sz in sp["f8"]:
                    seq.append(("d1", len(seq), so, j0, sz))
                for so, j0, sz in sp["i8"]:
                    seq.append(("s", len(seq), so, j0, sz))

            # -- sequential estimate pass: every writer lands before its
            # readers; each slot ring's writer k+bufs after reads of k --
            h_ready = [1.0 + H_DT * (q + 1) for q in range(nch)]
            mm_est = []
            wire_c = 0.5
            conv_busy = 6.0
            d_hist = []  # seq indices of direct tiles (atp ring order)
            s_hist = []  # seq indices of staged tiles (stg/raw rings)
            conv_ends = []  # conv completion est per staged tile
            for si, (kind, key, so, j0, sz) in enumerate(seq):
                if kind in ("d0", "d1"):
                    seg = 0 if kind == "d0" else 1
                    t = wire_c
                    if len(d_hist) >= at_bufs:
                        t = max(t, mm_est[d_hist[-at_bufs]] + 0.06)
                    eng_i = 0 if seg == 0 else 2
                    ev(t, emit_direct(key, seg, so, sz, eng_i))
                    wire_c = t + mbytes(sz, 2 if seg == 0 else 1) * WIRE
                    avail = t + 0.1
                    d_hist.append(si)
                else:
                    raw_t = max(conv_busy - 5.0, 0.06)
                    conv_t = conv_busy
                    if len(s_hist) >= RAW_BUFS:
                        # raw slot reader is the RAW_BUFS-back conversion
                        raw_t = max(raw_t, conv_ends[-RAW_BUFS] + 0.02)
                    if len(s_hist) >= STG_BUFS:
                        # stage slot reader is the STG_BUFS-back matmul
                        conv_t = max(
                            conv_t, mm_est[s_hist[-STG_BUFS]] + 0.06
                        )
                    ev(raw_t, emit_rawload(key, so, sz))
                    ev(conv_t, emit_conv(key, sz))
                    conv_busy = conv_t + CONV_US * sz + 0.3
                    conv_ends.append(conv_busy)
                    avail = conv_busy + 0.05
                    s_hist.append(si)
                q_last = m2ch[j0 + sz - 1][0]
                t = max(mm_est[-1] + 0.4 if mm_est else 0.4,
                        avail, h_ready[q_last] + 0.05)
                mm_est.append(t)

            def mmtile(si, final):
                kind, key, so, j0, sz = seq[si]
                def go(kind=kind, key=key, j0=j0, sz=sz, final=final):
                    tile_ap = at_tiles[key] if kind != "s" else stg_tiles[key]
                    if not final:
                        for kk in range(sz):
                            m = j0 + kk
                            q, jl = m2ch[m]
                            for t in range(nt_n):
                                nc.tensor.matmul(
                                    acc[t][:, :], hqs[q][:, jl, :],
                                    tile_ap[:, kk, t * ntc : (t + 1) * ntc],
                                    start=(m == 0), stop=False,
                                )
                    else:
                        # Final tile: t-outer so each acc closes in turn;
                        # drain to SBUF with fused bias add and store while
                        # the PE still works the later t slices.
                        for t in range(nt_n):
                            for kk in range(sz):
                                m = j0 + kk
                                q, jl = m2ch[m]
                                nc.tensor.matmul(
                                    acc[t][:, :], hqs[q][:, jl, :],
                                    tile_ap[:, kk, t * ntc : (t + 1) * ntc],
                                    start=(m == 0), stop=(kk == sz - 1),
                                )
                            nc.vector.tensor_add(
                                outT[:, t * ntc : (t + 1) * ntc],
                                acc[t][:, :],
                                biasT[:, t * ntc : (t + 1) * ntc],
                            )
                            eng2 = nc.scalar if t % 2 else nc.sync
                            eng2.dma_start(
                                out=outs.ap()[:, t * ntc : (t + 1) * ntc],
                                in_=outT[:, t * ntc : (t + 1) * ntc],
                            )
                return go

            for si in range(len(seq)):
                ev(mm_est[si], mmtile(si, si == len(seq) - 1))

            # -- emit everything in estimated-time order --
            events.sort(key=lambda e: (e[0], e[1]))
            for _, _, fn in events:
                fn()

    nc.compile()
    return nc


_NC_CACHE = {}


def _get_nc(n=N, ncores=NCORES):
    key = (n, ncores)
    if key not in _NC_CACHE:
        _NC_CACHE[key] = build_nc(n, ncores)
    return _NC_CACHE[key]


def make_in_maps(x, A, kern, bias, n=N, ncores=NCORES):
    import ml_dtypes

    nl = n // ncores
    jn = n // P
    f16_js, f8_js, i8_js, _ = _span_plan(jn)

    # Shared across cores: x and kernel in [p-major] DVE-friendly layouts.
    # xq[p, j, b, f] = x[b, j*P+p, f];  kq[p, j, g, f] = kern[j*P+p, f, g]
    x16 = x.astype(np.float16).transpose(1, 0, 2).reshape(jn, P, B, F)
    xq = np.ascontiguousarray(x16.transpose(1, 0, 2, 3)).reshape(P, jn * B * F)
    k16 = kern.astype(np.float16).transpose(0, 2, 1).reshape(jn, P, G, F)
    kq = np.ascontiguousarray(k16.transpose(1, 0, 2, 3)).reshape(P, jn * G * F)

    in_maps = []
    for r in range(ncores):
        sl = slice(r * nl, (r + 1) * nl)
        # Acol[j, p, :] = A^T[j*P + p, r-shard] (fp32, cast per region)
        Acol = A[sl, :].T.reshape(jn, P, nl)
        at16 = np.ascontiguousarray(
            Acol[f16_js].transpose(1, 0, 2)
        ).astype(ml_dtypes.bfloat16)
        at8 = np.ascontiguousarray(
            Acol[f8_js].transpose(1, 0, 2)
        ).astype(ml_dtypes.float8_e4m3)
        ati = np.clip(
            np.round(
                np.ascontiguousarray(Acol[i8_js].transpose(1, 0, 2)) / ISCALE
            ), -127, 127,
        ).astype(np.int8)
        # bsT[(b g), nl] = bias[shard][nl, g] for both b
        bT = np.ascontiguousarray(bias[sl].T)  # [G, nl]
        bsT = np.ascontiguousarray(np.tile(bT, (B, 1))).astype(np.float16)
        in_maps.append({
            "at16": at16, "at8": at8, "ati": ati,
            "xq": xq, "kq": kq, "bsT": bsT,
        })
    return in_maps


def assemble_out(results, n=N, ncores=NCORES):
    nl = n // ncores
    parts = []
    for r in range(ncores):
        o = results[r]["outs"].reshape(B, G, nl)
        parts.append(o.transpose(0, 2, 1))  # [B, nl, G]
    return np.ascontiguousarray(np.concatenate(parts, axis=1))


def run(inputs, n=N, ncores=NCORES, trace=False, **spmd_kwargs):
    from concourse.bass_utils import run_bass_kernel_spmd

    x = np.asarray(inputs["x"], dtype=np.float32)
    A = np.asarray(inputs["A"], dtype=np.float32)
    kern = np.asarray(inputs["kernel"], dtype=np.float32)
    bias = np.asarray(inputs["bias"], dtype=np.float32)
    nc = _get_nc(n, ncores)
    in_maps = make_in_maps(x, A, kern, bias, n, ncores)
    res = run_bass_kernel_spmd(
        nc, in_maps, list(range(ncores)), trace=trace, **spmd_kwargs
    )
    out = assemble_out(res.results, n, ncores)
    return out, res


def kernel(**inputs) -> np.ndarray:
    out, _ = run(inputs)
    return out


# revision 31
# speedup vs baseline: 1.0285x; 1.0285x over previous
"""Trainium2 Bass kernel for GCNN operator:
    h   = einsum('bnf,nfg->bng', x, kernel)   # per-node feature transform
    out = einsum('nm,bmg->bng', A, h) + bias  # dense adjacency aggregation

Sharding: node dim N row-sharded across 8 cores for the A@h matmul only.
Every core redundantly computes the FULL h on its DVE (x and kernel are
small), so there is NO collective — no cross-core barrier, no skew
sensitivity. The A-shard (pre-transposed, pre-tiled on host) streams from
HBM while the TensorEngine accumulates out^T = sum_m H_m^T @ A^T_m.

The kernel is HBM-bandwidth bound on the A stream, so A is stored in
three precisions, interleaved in SPANS of 16 j-blocks:
  per span: [4 x bf16 | 5 x fp8e4 | 7 x int8 @ scale 2^-5]
bf16/fp8 tiles feed the PE directly as the moving operand (the bf16/fp8
ISA path allows 1024-row matmuls, halving PE instruction count vs fp16;
the stationary h stays fp16). int8 j-blocks are converted to bf16 by the
Activation engine (hw-measured ~1.85us/j-block; GpSimd's software path
is 10x slower and is NOT used); int * 2^-5 is exact in bf16 so the
dequant scale rides in the conversion. The fine interleave is the load-
balancing trick: the PE walks j ascending, so conversions, h chunks,
DMA arrivals and PE consumption all advance uniformly — no engine ever
waits long on a buffer-recycle semaphore (a contiguous i8 region would
stall the converter on stage-slot reuse for ~100us).

Queues: SP carries x/bias + the bf16 tiles + stores; the GpSimd queue
carries the kq chunk stream (deadline-paced for the DVE's h chunks) +
the fp8 tiles; the Activation queue carries the int8 raws, self-paced
between conversions. A few warm-up matmuls on the x tile start the PE
p-state ramp early.

Emission goes through one global event list ordered by estimated time;
the estimates only shape ORDER (writer-before-reader, slot-ring WAR
pairing, and trigger/compute interleave per engine).

Self-contained: hardcodes shapes; only imports concourse + numpy.
"""

import numpy as np

B, N, F, G = 2, 16384, 16, 16
NCORES = 8
P = 128                    # SBUF partitions
C = B * G                  # 32 fused (batch, out-feature) columns
NT = 512                   # matmul moving free-dim (ISA AP element cap)
SPAN = 16                  # j-blocks per precision span
SP16 = 4                   # bf16 j-blocks per span
SP8 = 5                    # fp8 j-blocks per span
ITILE = (2, 2, 3)          # int8 tile sizes within a span
JJ = 8                     # j-blocks per H compute chunk
AT_BUFS = 4                # direct A-stream buffering depth
RAW_BUFS = 3               # int8 raw ring depth
STG_BUFS = 4               # converted-tile ring depth
KS_BUFS = 6                # kq chunk ring depth
ISCALE = 2.0 ** -5         # int8 quantization scale
N_PRIME = 8                # PE warm-up matmuls


def _span_plan(jn):
    """Per-j-block encoding plan. Returns (f16_js, f8_js, i8_js, spans)
    where spans is a list of per-span dicts with tile descriptors."""
    if jn >= SPAN:
        nspan = jn // SPAN
        sp16, sp8 = SP16, SP8
    else:
        nspan = 1
        sp16 = max((jn * 2 * SP16) // (2 * SPAN), 1)
        sp8 = max((jn * 2 * SP8) // (2 * SPAN), 1)
    f16_js, f8_js, i8_js, spans = [], [], [], []
    for s in range(nspan):
        base = s * (jn // nspan)
        width = jn // nspan
        n16 = sp16
        n8 = sp8
        ni = width - n16 - n8
        sp = {"f16": [], "f8": [], "i8": []}
        js16 = list(range(base, base + n16))
        js8 = list(range(base + n16, base + n16 + n8))
        jsi = list(range(base + n16 + n8, base + width))
        sp["f16"].append((len(f16_js), js16[0], n16))
        f16_js += js16
        sp["f8"].append((len(f8_js), js8[0], n8))
        f8_js += js8
        o = 0
        it = list(ITILE)
        while o < ni:
            sz = it.pop(0) if it else ni - o
            sz = min(sz, ni - o)
            sp["i8"].append((len(i8_js) + o, jsi[0] + o, sz))
            o += sz
        i8_js += jsi
        spans.append(sp)
    return f16_js, f8_js, i8_js, spans


def build_nc(n=N, ncores=NCORES, at_bufs=AT_BUFS):
    """Build the per-core Bass program (SPMD: same program on all cores)."""
    import concourse.bass as bass
    import concourse.mybir as mybir
    import concourse.tile as tile
    from concourse import bacc

    f32 = mybir.dt.float32
    f16 = mybir.dt.float16
    bf16 = mybir.dt.bfloat16
    f8 = mybir.dt.float8e4
    i8 = mybir.dt.int8

    nl = n // ncores           # local output rows per core
    jn = n // P                # contraction j-blocks over FULL n
    f16_js, f8_js, i8_js, spans = _span_plan(jn)
    n16, n8, ni = len(f16_js), len(f8_js), len(i8_js)
    ntc = min(NT, nl)          # matmul moving free-dim
    nt_n = max(nl // ntc, 1)   # acc tiles

    # H chunks of JJ j-blocks
    jj = min(JJ, jn)
    ch_sizes = [jj] * (jn // jj)
    if jn % jj:
        ch_sizes.append(jn % jj)
    nch = len(ch_sizes)
    ch_off = [0]
    for sz in ch_sizes:
        ch_off.append(ch_off[-1] + sz)
    m2ch = []
    for i, sz in enumerate(ch_sizes):
        for jl in range(sz):
            m2ch.append((i, jl))

    nc = bacc.Bacc("TRN2", target_bir_lowering=False, debug=False, num_devices=1)

    at16 = nc.dram_tensor("at16", [P, n16, nl], bf16, kind="ExternalInput")
    at8 = nc.dram_tensor("at8", [P, n8, nl], f8, kind="ExternalInput")
    ati = nc.dram_tensor("ati", [P, ni, nl], i8, kind="ExternalInput")
    xq = nc.dram_tensor("xq", [P, jn * B * F], f16, kind="ExternalInput")
    kq = nc.dram_tensor("kq", [P, jn * G * F], f16, kind="ExternalInput")
    bsT = nc.dram_tensor("bsT", [C, nl], f16, kind="ExternalInput")
    outs = nc.dram_tensor("outs", [C, nl], f32, kind="ExternalOutput")

    kq_r = kq.ap().rearrange("p (j g f) -> p j g f", g=G, f=F)

    # ---- timing estimates (µs), used only for emission ordering ----
    H_DT = 7.3                 # µs per DVE h chunk (2 bcast mults + 2 reduces)
    CONV_US = 1.9 * nl / 2048  # µs per converted j-block on ACT
    WIRE = 2.6                 # aggregate stream µs per MB
    mbytes = lambda nblk, bpe: P * nl * nblk * bpe / 1e6

    events = []

    def ev(t, fn):
        events.append((t, len(events), fn))

    with tile.TileContext(nc) as tc:
        with (
            tc.tile_pool(name="const", bufs=1) as const,
            tc.tile_pool(name="work", bufs=2) as work,
            tc.tile_pool(name="ksp", bufs=KS_BUFS) as ksp,
            tc.tile_pool(name="atp", bufs=at_bufs) as atp,
            tc.tile_pool(name="rawp", bufs=RAW_BUFS) as rawp,
            tc.tile_pool(name="stgp", bufs=STG_BUFS) as stgp,
            tc.tile_pool(name="pacc", bufs=1, space="PSUM") as pacc,
        ):
            xs = const.tile([P, jn, B, F], f16)
            biasT = const.tile([C, nl], f16)
            ks_tiles = [None] * nch
            hqs = [
                const.tile([P, ch_sizes[q], C], f16, tag=f"hq{q}",
                           name=f"hq{q}")
                for q in range(nch)
            ]
            outT = work.tile([C, nl], f32, tag="outT")
            acc = [
                pacc.tile([C, ntc], f32, tag=f"acc{t}", name=f"acc{t}")
                for t in range(nt_n)
            ]
            pw = min(512, jn * B * F)
            pprime = pacc.tile([C, pw], f32, tag="pprime")
            xs_flat = xs.rearrange("p a b c -> p (a b c)")

            # -- prologue on the sync queue: ks0 rides FIRST so the DVE's
            # first h chunk can start at ~7us, then xs, then bias --
            def load_ks0():
                kt = ksp.tile([P, jj, G, F], f16, tag="ks", name="ks0")
                nc.sync.dma_start(
                    out=kt[:, : ch_sizes[0], :, :], in_=kq_r[:, 0 : ch_off[1]]
                )
                ks_tiles[0] = kt
            ev(0.0, load_ks0)
            ev(0.005, lambda: nc.sync.dma_start(
                out=xs[:, :, :, :],
                in_=xq.ap().rearrange("p (j b f) -> p j b f", b=B, f=F),
            ))
            ev(0.01, lambda: nc.sync.dma_start(out=biasT[:, :], in_=bsT.ap()))

            # -- PE warm-up: p-state ramp starts once x lands --
            def prime():
                for _ in range(N_PRIME):
                    nc.tensor.matmul(
                        pprime[:, :], xs_flat[:, 0:C], xs_flat[:, 0:pw],
                        start=True, stop=True,
                    )
            ev(0.02, prime)

            # -- kq chunk ring on the gpsimd queue: a KS_BUFS-deep ring
            # paced just ahead of the DVE's h chunks (the slot WAR pairs
            # load q with the DVE's reads of chunk q-KS_BUFS) --
            for q in range(1, nch):
                def load_ks(q=q):
                    kt = ksp.tile([P, jj, G, F], f16, tag="ks",
                                  name=f"ks{q}")
                    nc.gpsimd.dma_start(
                        out=kt[:, : ch_sizes[q], :, :],
                        in_=kq_r[:, ch_off[q] : ch_off[q + 1]],
                    )
                    ks_tiles[q] = kt
                if q < 4:
                    t = 0.03 + 0.01 * q
                else:
                    t = 1.5 + H_DT * (q - 4)
                ev(t, load_ks)

            # -- h chunks on the DVE (per-b broadcast mult + reduce; the
            # single-broadcast mult hits the DVE's packed 2x mode; NOTE:
            # GpSimd must NOT help here — it shares an SBUF port with the
            # DVE under an exclusive lock and serializes both) --
            for q in range(nch):
                sz = ch_sizes[q]
                def hstep(q=q, sz=sz):
                    with nc.allow_low_precision(
                        reason="h accum over F=16 in fp16"
                    ):
                        for b in range(B):
                            prod = work.tile([P, jj, G, F], f16, tag="prod")
                            nc.vector.tensor_tensor(
                                prod[:, :sz, :, :],
                                xs[
                                    :, ch_off[q] : ch_off[q + 1], b, None, :
                                ].to_broadcast([P, sz, G, F]),
                                ks_tiles[q][:, :sz, :, :],
                                mybir.AluOpType.mult,
                            )
                            nc.vector.tensor_reduce(
                                hqs[q][:, :, b * G : (b + 1) * G],
                                prod[:, :sz, :, :],
                                axis=mybir.AxisListType.X,
                                op=mybir.AluOpType.add,
                            )
                ev(1.0 + H_DT * q, hstep)

            # -- per-tile emitters --
            at_tiles = {}
            raw_tiles = {}
            stg_tiles = {}

            def emit_direct(key, seg, so, sz, eng_i):
                def trig():
                    dt_t = bf16 if seg == 0 else f8
                    src = at16 if seg == 0 else at8
                    at_t = atp.tile([P, sz, nl], dt_t, tag="at_t",
                                    name="at_t")
                    eng = [nc.sync, nc.scalar, nc.gpsimd][eng_i]
                    eng.dma_start(
                        out=at_t[:, :, :], in_=src.ap()[:, so : so + sz, :]
                    )
                    at_tiles[key] = at_t
                return trig

            def emit_rawload(key, so, sz):
                def rawload():
                    rt = rawp.tile([P, max(ITILE), nl], i8, tag="rawt",
                                   name=f"raw{key}")
                    nc.scalar.dma_start(
                        out=rt[:, :sz, :], in_=ati.ap()[:, so : so + sz, :]
                    )
                    raw_tiles[key] = rt
                return rawload

            def emit_conv(key, sz):
                def conv():
                    rt = raw_tiles[key]
                    st = stgp.tile([P, max(ITILE), nl], bf16, tag="stgt",
                                   name=f"stg{key}")
                    with nc.allow_low_precision(
                        reason="int8*2^-5 dequant is exact in bf16"
                    ):
                        nc.scalar.activation(
                            st[:, :sz, :], rt[:, :sz, :],
                            mybir.ActivationFunctionType.Copy, scale=ISCALE,
                        )
                    stg_tiles[key] = st
                return conv

            # -- build the PE consumption sequence: span by span, j
            # ascending: f16 tile, f8 tile, then the span's i8 tiles --
            seq = []  # (kind, key, seg_off, abs_j0, sz)
            for sp in spans:
                for so, j0, sz in sp["f16"]:
                    seq.append(("d0", len(seq), so, j0, sz))
                for so, j0, sz in sp["f8"]:
                    seq.append(("d1", len(seq), so, j0, sz))
                for so, j0, sz in sp["i8"]:
                    seq.append(("s", len(seq), so, j0, sz))

            # -- sequential estimate pass: every writer lands before its
            # readers; each slot ring's writer k+bufs after reads of k --
            h_ready = [1.0 + H_DT * (q + 1) for q in range(nch)]
            mm_est = []
            wire_c = 0.5
            conv_busy = 6.0
            d_hist = []  # seq indices of direct tiles (atp ring order)
            s_hist = []  # seq indices of staged tiles (stg/raw rings)
            conv_ends = []  # conv completion est per staged tile
            for si, (kind, key, so, j0, sz) in enumerate(seq):
                if kind in ("d0", "d1"):
                    seg = 0 if kind == "d0" else 1
                    t = wire_c
                    if len(d_hist) >= at_bufs:
                        t = max(t, mm_est[d_hist[-at_bufs]] + 0.06)
                    eng_i = 0 if seg == 0 else 2
                    ev(t, emit_direct(key, seg, so, sz, eng_i))
                    wire_c = t + mbytes(sz, 2 if seg == 0 else 1) * WIRE
                    avail = t + 0.1
                    d_hist.append(si)
                else:
                    raw_t = max(conv_busy - 5.0, 0.06)
                    conv_t = conv_busy
                    if len(s_hist) >= RAW_BUFS:
                        # raw slot reader is the RAW_BUFS-back conversion
                        raw_t = max(raw_t, conv_ends[-RAW_BUFS] + 0.02)
                    if len(s_hist) >= STG_BUFS:
                        # stage slot reader is the STG_BUFS-back matmul
                        conv_t = max(
                            conv_t, mm_est[s_hist[-STG_BUFS]] + 0.06
                        )
                    ev(raw_t, emit_rawload(key, so, sz))
                    ev(conv_t, emit_conv(key, sz))
                    conv_busy = conv_t + CONV_US * sz + 0.3
                    conv_ends.append(conv_busy)
                    avail = conv_busy + 0.05
                    s_hist.append(si)
                q_last = m2ch[j0 + sz - 1][0]
                t = max(mm_est[-1] + 0.4 if mm_est else 0.4,
                        avail, h_ready[q_last] + 0.05)
                mm_est.append(t)

            def mmtile(si, final):
                kind, key, so, j0, sz = seq[si]
                def go(kind=kind, key=key, j0=j0, sz=sz, final=final):
                    tile_ap = at_tiles[key] if kind != "s" else stg_tiles[key]
                    if not final:
                        for kk in range(sz):
                            m = j0 + kk
                            q, jl = m2ch[m]
                            for t in range(nt_n):
                                nc.tensor.matmul(
                                    acc[t][:, :], hqs[q][:, jl, :],
                                    tile_ap[:, kk, t * ntc : (t + 1) * ntc],
                                    start=(m == 0), stop=False,
                                )
                    else:
                        # Final tile: t-outer so each acc closes in turn;
                        # drain to SBUF with fused bias add and store while
                        # the PE still works the later t slices.
                        for t in range(nt_n):
                            for kk in range(sz):
                                m = j0 + kk
                                q, jl = m2ch[m]
                                nc.tensor.matmul(
                                    acc[t][:, :], hqs[q][:, jl, :],
                                    tile_ap[:, kk, t * ntc : (t + 1) * ntc],
                                    start=(m == 0), stop=(kk == sz - 1),
                                )
                            nc.vector.tensor_add(
                                outT[:, t * ntc : (t + 1) * ntc],
                                acc[t][:, :],
                                biasT[:, t * ntc : (t + 1) * ntc],
                            )
                            eng2 = nc.scalar if t % 2 else nc.sync
                            eng2.dma_start(
                                out=outs.ap()[:, t * ntc : (t + 1) * ntc],
                                in_=outT[:, t * ntc : (t + 1) * ntc],
                            )
                return go

            for si in range(len(seq)):
                ev(mm_est[si], mmtile(si, si == len(seq) - 1))

            # -- emit everything in estimated-time order --
            events.sort(key=lambda e: (e[0], e[1]))
            for _, _, fn in events:
                fn()

    nc.compile()
    return nc


_NC_CACHE = {}


def _get_nc(n=N, ncores=NCORES):
    key = (n, ncores)
    if key not in _NC_CACHE:
        _NC_CACHE[key] = build_nc(n, ncores)
    return _NC_CACHE[key]


def make_in_maps(x, A, kern, bias, n=N, ncores=NCORES):
    import ml_dtypes

    nl = n // ncores
    jn = n // P
    f16_js, f8_js, i8_js, _ = _span_plan(jn)

    # Shared across cores: x and kernel in [p-major] DVE-friendly layouts.
    # xq[p, j, b, f] = x[b, j*P+p, f];  kq[p, j, g, f] = kern[j*P+p, f, g]
    x16 = x.astype(np.float16).transpose(1, 0, 2).reshape(jn, P, B, F)
    xq = np.ascontiguousarray(x16.transpose(1, 0, 2, 3)).reshape(P, jn * B * F)
    k16 = kern.astype(np.float16).transpose(0, 2, 1).reshape(jn, P, G, F)
    kq = np.ascontiguousarray(k16.transpose(1, 0, 2, 3)).reshape(P, jn * G * F)

    in_maps = []
    for r in range(ncores):
        sl = slice(r * nl, (r + 1) * nl)
        # Acol[j, p, :] = A^T[j*P + p, r-shard] (fp32, cast per region)
        Acol = A[sl, :].T.reshape(jn, P, nl)
        at16 = np.ascontiguousarray(
            Acol[f16_js].transpose(1, 0, 2)
        ).astype(ml_dtypes.bfloat16)
        at8 = np.ascontiguousarray(
            Acol[f8_js].transpose(1, 0, 2)
        ).astype(ml_dtypes.float8_e4m3)
        ati = np.clip(
            np.round(
                np.ascontiguousarray(Acol[i8_js].transpose(1, 0, 2)) / ISCALE
            ), -127, 127,
        ).astype(np.int8)
        # bsT[(b g), nl] = bias[shard][nl, g] for both b
        bT = np.ascontiguousarray(bias[sl].T)  # [G, nl]
        bsT = np.ascontiguousarray(np.tile(bT, (B, 1))).astype(np.float16)
        in_maps.append({
            "at16": at16, "at8": at8, "ati": ati,
            "xq": xq, "kq": kq, "bsT": bsT,
        })
    return in_maps


def assemble_out(results, n=N, ncores=NCORES):
    nl = n // ncores
    parts = []
    for r in range(ncores):
        o = results[r]["outs"].reshape(B, G, nl)
        parts.append(o.transpose(0, 2, 1))  # [B, nl, G]
    return np.ascontiguousarray(np.concatenate(parts, axis=1))


def run(inputs, n=N, ncores=NCORES, trace=False, **spmd_kwargs):
    from concourse.bass_utils import run_bass_kernel_spmd

    x = np.asarray(inputs["x"], dtype=np.float32)
    A = np.asarray(inputs["A"], dtype=np.float32)
    kern = np.asarray(inputs["kernel"], dtype=np.float32)
    bias = np.asarray(inputs["bias"], dtype=np.float32)
    nc = _get_nc(n, ncores)
    in_maps = make_in_maps(x, A, kern, bias, n, ncores)
    res = run_bass_kernel_spmd(
        nc, in_maps, list(range(ncores)), trace=trace, **spmd_kwargs
    )
    out = assemble_out(res.results, n, ncores)
    return out, res


def kernel(**inputs) -> np.ndarray:
    out, _ = run(inputs)
    return out


# revision 35
# speedup vs baseline: 1.0735x; 1.0437x over previous
"""Trainium2 Bass kernel for GCNN operator:
    h   = einsum('bnf,nfg->bng', x, kernel)   # per-node feature transform
    out = einsum('nm,bmg->bng', A, h) + bias  # dense adjacency aggregation

Sharding: node dim N row-sharded across 8 cores for the A@h matmul only.
Every core redundantly computes the FULL h on its DVE (x and kernel are
small), so there is NO collective — no cross-core barrier, no skew
sensitivity. The A-shard (pre-transposed, pre-tiled on host) streams from
HBM while the TensorEngine accumulates out^T = sum_m H_m^T @ A^T_m.

The kernel is HBM-bandwidth bound on the A stream, so A is stored in
three precisions, interleaved in SPANS of 16 j-blocks:
  per span: [4 x bf16 | 5 x fp8e4 | 7 x int8 @ scale 2^-5]
bf16/fp8 tiles feed the PE directly as the moving operand (the bf16/fp8
ISA path allows 1024-row matmuls, halving PE instruction count vs fp16;
the stationary h stays fp16). int8 j-blocks are converted to bf16 by the
Activation engine (hw-measured ~1.85us/j-block; GpSimd's software path
is 10x slower and is NOT used); int * 2^-5 is exact in bf16 so the
dequant scale rides in the conversion. The fine interleave is the load-
balancing trick: the PE walks j ascending, so conversions, h chunks,
DMA arrivals and PE consumption all advance uniformly — no engine ever
waits long on a buffer-recycle semaphore (a contiguous i8 region would
stall the converter on stage-slot reuse for ~100us).

Queues: SP carries x/bias + the bf16 tiles + stores; the GpSimd queue
carries the kq chunk stream (deadline-paced for the DVE's h chunks) +
the fp8 tiles; the Activation queue carries the int8 raws, self-paced
between conversions. A few warm-up matmuls on the x tile start the PE
p-state ramp early.

Emission goes through one global event list ordered by estimated time;
the estimates only shape ORDER (writer-before-reader, slot-ring WAR
pairing, and trigger/compute interleave per engine).

Self-contained: hardcodes shapes; only imports concourse + numpy.
"""

import numpy as np

B, N, F, G = 2, 16384, 16, 16
NCORES = 8
P = 128                    # SBUF partitions
C = B * G                  # 32 fused (batch, out-feature) columns
NT = 512                   # matmul moving free-dim (ISA AP element cap)
SPAN = 16                  # j-blocks per precision span
SP16 = 4                   # bf16 j-blocks per span
SP8 = 5                    # fp8 j-blocks per span
ITILE = (2, 2, 3)          # int8 tile sizes within a span
JJ = 8                     # j-blocks per H compute chunk
AT_BUFS = 4                # direct A-stream buffering depth
RAW_BUFS = 3               # int8 raw ring depth
STG_BUFS = 4               # converted-tile ring depth
KS_BUFS = 6                # kq chunk ring depth
ISCALE = 2.0 ** -5         # int8 quantization scale
N_PRIME = 8                # PE warm-up matmuls


def _span_plan(jn):
    """Per-j-block encoding plan. Returns (f16_js, f8_js, i8_js, spans)
    where spans is a list of per-span dicts with tile descriptors."""
    if jn >= SPAN:
        nspan = jn // SPAN
        sp16, sp8 = SP16, SP8
    else:
        nspan = 1
        sp16 = max((jn * 2 * SP16) // (2 * SPAN), 1)
        sp8 = max((jn * 2 * SP8) // (2 * SPAN), 1)
    f16_js, f8_js, i8_js, spans = [], [], [], []
    for s in range(nspan):
        base = s * (jn // nspan)
        width = jn // nspan
        n16 = sp16
        n8 = sp8
        ni = width - n16 - n8
        sp = {"f16": [], "f8": [], "i8": []}
        js16 = list(range(base, base + n16))
        js8 = list(range(base + n16, base + n16 + n8))
        jsi = list(range(base + n16 + n8, base + width))
        sp["f16"].append((len(f16_js), js16[0], n16))
        f16_js += js16
        sp["f8"].append((len(f8_js), js8[0], n8))
        f8_js += js8
        o = 0
        it = list(ITILE)
        while o < ni:
            sz = it.pop(0) if it else ni - o
            sz = min(sz, ni - o)
            sp["i8"].append((len(i8_js) + o, jsi[0] + o, sz))
            o += sz
        i8_js += jsi
        spans.append(sp)
    return f16_js, f8_js, i8_js, spans


def build_nc(n=N, ncores=NCORES, at_bufs=AT_BUFS):
    """Build the per-core Bass program (SPMD: same program on all cores)."""
    import concourse.bass as bass
    import concourse.mybir as mybir
    import concourse.tile as tile
    from concourse import bacc

    f32 = mybir.dt.float32
    f16 = mybir.dt.float16
    bf16 = mybir.dt.bfloat16
    f8 = mybir.dt.float8e4
    i8 = mybir.dt.int8

    nl = n // ncores           # local output rows per core
    jn = n // P                # contraction j-blocks over FULL n
    f16_js, f8_js, i8_js, spans = _span_plan(jn)
    n16, n8, ni = len(f16_js), len(f8_js), len(i8_js)
    ntc = min(NT, nl)          # matmul moving free-dim
    nt_n = max(nl // ntc, 1)   # acc tiles

    # H chunks of JJ j-blocks
    jj = min(JJ, jn)
    ch_sizes = [jj] * (jn // jj)
    if jn % jj:
        ch_sizes.append(jn % jj)
    nch = len(ch_sizes)
    ch_off = [0]
    for sz in ch_sizes:
        ch_off.append(ch_off[-1] + sz)
    m2ch = []
    for i, sz in enumerate(ch_sizes):
        for jl in range(sz):
            m2ch.append((i, jl))

    nc = bacc.Bacc("TRN2", target_bir_lowering=False, debug=False, num_devices=1)

    at16 = nc.dram_tensor("at16", [P, n16, nl], bf16, kind="ExternalInput")
    at8 = nc.dram_tensor("at8", [P, n8, nl], f8, kind="ExternalInput")
    ati = nc.dram_tensor("ati", [P, ni, nl], i8, kind="ExternalInput")
    xq = nc.dram_tensor("xq", [P, jn * B * F], f16, kind="ExternalInput")
    kq = nc.dram_tensor("kq", [P, jn * G * F], f16, kind="ExternalInput")
    bsT = nc.dram_tensor("bsT", [C, nl], f16, kind="ExternalInput")
    outs = nc.dram_tensor("outs", [C, nl], f32, kind="ExternalOutput")

    kq_r = kq.ap().rearrange("p (j g f) -> p j g f", g=G, f=F)

    # ---- timing estimates (µs), used only for emission ordering ----
    H_DT = 7.3                 # µs per DVE h chunk (2 bcast mults + 2 reduces)
    CONV_US = 1.9 * nl / 2048  # µs per converted j-block on ACT
    WIRE = 2.6                 # aggregate stream µs per MB
    mbytes = lambda nblk, bpe: P * nl * nblk * bpe / 1e6

    events = []

    def ev(t, fn):
        events.append((t, len(events), fn))

    with tile.TileContext(nc) as tc:
        with (
            tc.tile_pool(name="const", bufs=1) as const,
            tc.tile_pool(name="work", bufs=2) as work,
            tc.tile_pool(name="ksp", bufs=KS_BUFS) as ksp,
            tc.tile_pool(name="atp", bufs=at_bufs) as atp,
            tc.tile_pool(name="rawp", bufs=RAW_BUFS) as rawp,
            tc.tile_pool(name="stgp", bufs=STG_BUFS) as stgp,
            tc.tile_pool(name="pacc", bufs=1, space="PSUM") as pacc,
        ):
            xs = const.tile([P, jn, B, F], f16)
            biasT = const.tile([C, nl], f16)
            ks_tiles = [None] * nch
            hqs = [
                const.tile([P, ch_sizes[q], C], f16, tag=f"hq{q}",
                           name=f"hq{q}")
                for q in range(nch)
            ]
            outT = work.tile([C, nl], f32, tag="outT")
            acc = [
                pacc.tile([C, ntc], f32, tag=f"acc{t}", name=f"acc{t}")
                for t in range(nt_n)
            ]
            pw = min(512, jn * B * F)
            pprime = pacc.tile([C, pw], f32, tag="pprime")
            xs_flat = xs.rearrange("p a b c -> p (a b c)")

            # -- prologue: xs first on the sync queue while ks0 rides the
            # gpsimd queue in parallel — both land ~11us, h starts ~13us --
            ev(0.0, lambda: nc.sync.dma_start(
                out=xs[:, :, :, :],
                in_=xq.ap().rearrange("p (j b f) -> p j b f", b=B, f=F),
            ))
            ev(0.01, lambda: nc.sync.dma_start(out=biasT[:, :], in_=bsT.ap()))

            # -- PE warm-up: p-state ramp starts once x lands --
            def prime():
                for _ in range(N_PRIME):
                    nc.tensor.matmul(
                        pprime[:, :], xs_flat[:, 0:C], xs_flat[:, 0:pw],
                        start=True, stop=True,
                    )
            ev(0.02, prime)

            # -- kq chunk ring: the gpsimd queue's TOP priority (nothing
            # big rides ahead of a due ks chunk — a late ks chunk stalls
            # the DVE h chain and with it the whole kernel) --
            for q in range(nch):
                def load_ks(q=q):
                    kt = ksp.tile([P, jj, G, F], f16, tag="ks",
                                  name=f"ks{q}")
                    nc.gpsimd.dma_start(
                        out=kt[:, : ch_sizes[q], :, :],
                        in_=kq_r[:, ch_off[q] : ch_off[q + 1]],
                    )
                    ks_tiles[q] = kt
                if q < 3:
                    t = 0.02 + 0.01 * q
                else:
                    t = H_DT * (q - 2)
                ev(t, load_ks)

            # -- h chunks on the DVE (per-b broadcast mult + reduce; the
            # single-broadcast mult hits the DVE's packed 2x mode; NOTE:
            # GpSimd must NOT help here — it shares an SBUF port with the
            # DVE under an exclusive lock and serializes both) --
            for q in range(nch):
                sz = ch_sizes[q]
                def hstep(q=q, sz=sz):
                    with nc.allow_low_precision(
                        reason="h accum over F=16 in fp16"
                    ):
                        for b in range(B):
                            prod = work.tile([P, jj, G, F], f16, tag="prod")
                            nc.vector.tensor_tensor(
                                prod[:, :sz, :, :],
                                xs[
                                    :, ch_off[q] : ch_off[q + 1], b, None, :
                                ].to_broadcast([P, sz, G, F]),
                                ks_tiles[q][:, :sz, :, :],
                                mybir.AluOpType.mult,
                            )
                            nc.vector.tensor_reduce(
                                hqs[q][:, :, b * G : (b + 1) * G],
                                prod[:, :sz, :, :],
                                axis=mybir.AxisListType.X,
                                op=mybir.AluOpType.add,
                            )
                ev(1.0 + H_DT * q, hstep)

            # -- per-tile emitters --
            at_tiles = {}
            raw_tiles = {}
            stg_tiles = {}

            def emit_direct(key, seg, so, sz, eng_i):
                def trig():
                    dt_t = bf16 if seg == 0 else f8
                    src = at16 if seg == 0 else at8
                    at_t = atp.tile([P, sz, nl], dt_t, tag="at_t",
                                    name="at_t")
                    eng = [nc.sync, nc.scalar, nc.gpsimd][eng_i]
                    eng.dma_start(
                        out=at_t[:, :, :], in_=src.ap()[:, so : so + sz, :]
                    )
                    at_tiles[key] = at_t
                return trig

            def emit_rawload(key, so, sz, eng_i):
                def rawload():
                    rt = rawp.tile([P, max(ITILE), nl], i8, tag="rawt",
                                   name=f"raw{key}")
                    eng = [nc.sync, nc.scalar, nc.gpsimd][eng_i]
                    eng.dma_start(
                        out=rt[:, :sz, :], in_=ati.ap()[:, so : so + sz, :]
                    )
                    raw_tiles[key] = rt
                return rawload

            def emit_conv(key, sz):
                def conv():
                    rt = raw_tiles[key]
                    st = stgp.tile([P, max(ITILE), nl], bf16, tag="stgt",
                                   name=f"stg{key}")
                    with nc.allow_low_precision(
                        reason="int8*2^-5 dequant is exact in bf16"
                    ):
                        nc.scalar.activation(
                            st[:, :sz, :], rt[:, :sz, :],
                            mybir.ActivationFunctionType.Copy, scale=ISCALE,
                        )
                    stg_tiles[key] = st
                return conv

            # -- build the PE consumption sequence: span by span, j
            # ascending: f16 tile, f8 tile, then the span's i8 tiles --
            seq = []  # (kind, key, seg_off, abs_j0, sz)
            for sp in spans:
                for so, j0, sz in sp["f16"]:
                    seq.append(("d0", len(seq), so, j0, sz))
                for so, j0, sz in sp["f8"]:
                    seq.append(("d1", len(seq), so, j0, sz))
                for so, j0, sz in sp["i8"]:
                    seq.append(("s", len(seq), so, j0, sz))

            # -- sequential estimate pass: every writer lands before its
            # readers; each slot ring's writer k+bufs after reads of k --
            h_ready = [1.0 + H_DT * (q + 1) for q in range(nch)]
            mm_est = []
            wire_c = 0.5
            conv_busy = 6.0
            d_hist = []  # seq indices of direct tiles (atp ring order)
            s_hist = []  # seq indices of staged tiles (stg/raw rings)
            conv_ends = []  # conv completion est per staged tile
            for si, (kind, key, so, j0, sz) in enumerate(seq):
                if kind in ("d0", "d1"):
                    seg = 0 if kind == "d0" else 1
                    t = wire_c
                    if len(d_hist) >= at_bufs:
                        t = max(t, mm_est[d_hist[-at_bufs]] + 0.06)
                    # f16 tiles alternate SP/ACT queues; f8 rides SP (the
                    # gpsimd queue is reserved for ks + late raws)
                    eng_i = (len(d_hist) % 2) if seg == 0 else 0
                    ev(t, emit_direct(key, seg, so, sz, eng_i))
                    wire_c = t + mbytes(sz, 2 if seg == 0 else 1) * WIRE
                    avail = t + 0.1
                    d_hist.append(si)
                else:
                    raw_t = max(conv_busy - 5.0, 0.06)
                    conv_t = conv_busy
                    if len(s_hist) >= RAW_BUFS:
                        # raw slot reader is the RAW_BUFS-back conversion
                        raw_t = max(raw_t, conv_ends[-RAW_BUFS] + 0.02)
                    if len(s_hist) >= STG_BUFS:
                        # stage slot reader is the STG_BUFS-back matmul
                        conv_t = max(
                            conv_t, mm_est[s_hist[-STG_BUFS]] + 0.06
                        )
                    # early raws ride the ACT queue, late ones gpsimd
                    # (free once the ks stream drains)
                    reng = 1 if len(s_hist) < 12 else 2
                    ev(raw_t, emit_rawload(key, so, sz, reng))
                    ev(conv_t, emit_conv(key, sz))
                    conv_busy = conv_t + CONV_US * sz + 0.3
                    conv_ends.append(conv_busy)
                    avail = conv_busy + 0.05
                    s_hist.append(si)
                q_last = m2ch[j0 + sz - 1][0]
                t = max(mm_est[-1] + 0.4 if mm_est else 0.4,
                        avail, h_ready[q_last] + 0.05)
                mm_est.append(t)

            def mmtile(si, final):
                kind, key, so, j0, sz = seq[si]
                def go(kind=kind, key=key, j0=j0, sz=sz, final=final):
                    tile_ap = at_tiles[key] if kind != "s" else stg_tiles[key]
                    if not final:
                        for kk in range(sz):
                            m = j0 + kk
                            q, jl = m2ch[m]
                            for t in range(nt_n):
                                nc.tensor.matmul(
                                    acc[t][:, :], hqs[q][:, jl, :],
                                    tile_ap[:, kk, t * ntc : (t + 1) * ntc],
                                    start=(m == 0), stop=False,
                                )
                    else:
                        # Final tile: t-outer so each acc closes in turn;
                        # drain to SBUF with fused bias add and store while
                        # the PE still works the later t slices.
                        for t in range(nt_n):
                            for kk in range(sz):
                                m = j0 + kk
                                q, jl = m2ch[m]
                                nc.tensor.matmul(
                                    acc[t][:, :], hqs[q][:, jl, :],
                                    tile_ap[:, kk, t * ntc : (t + 1) * ntc],
                                    start=(m == 0), stop=(kk == sz - 1),
                                )
                            nc.vector.tensor_add(
                                outT[:, t * ntc : (t + 1) * ntc],
                                acc[t][:, :],
                                biasT[:, t * ntc : (t + 1) * ntc],
                            )
                            eng2 = nc.scalar if t % 2 else nc.sync
                            eng2.dma_start(
                                out=outs.ap()[:, t * ntc : (t + 1) * ntc],
                                in_=outT[:, t * ntc : (t + 1) * ntc],
                            )
                return go

            for si in range(len(seq)):
                ev(mm_est[si], mmtile(si, si == len(seq) - 1))

            # -- emit everything in estimated-time order --
            events.sort(key=lambda e: (e[0], e[1]))
            for _, _, fn in events:
                fn()

    nc.compile()
    return nc


_NC_CACHE = {}


def _get_nc(n=N, ncores=NCORES):
    key = (n, ncores)
    if key not in _NC_CACHE:
        _NC_CACHE[key] = build_nc(n, ncores)
    return _NC_CACHE[key]


def make_in_maps(x, A, kern, bias, n=N, ncores=NCORES):
    import ml_dtypes

    nl = n // ncores
    jn = n // P
    f16_js, f8_js, i8_js, _ = _span_plan(jn)

    # Shared across cores: x and kernel in [p-major] DVE-friendly layouts.
    # xq[p, j, b, f] = x[b, j*P+p, f];  kq[p, j, g, f] = kern[j*P+p, f, g]
    x16 = x.astype(np.float16).transpose(1, 0, 2).reshape(jn, P, B, F)
    xq = np.ascontiguousarray(x16.transpose(1, 0, 2, 3)).reshape(P, jn * B * F)
    k16 = kern.astype(np.float16).transpose(0, 2, 1).reshape(jn, P, G, F)
    kq = np.ascontiguousarray(k16.transpose(1, 0, 2, 3)).reshape(P, jn * G * F)

    in_maps = []
    for r in range(ncores):
        sl = slice(r * nl, (r + 1) * nl)
        # Acol[j, p, :] = A^T[j*P + p, r-shard] (fp32, cast per region)
        Acol = A[sl, :].T.reshape(jn, P, nl)
        at16 = np.ascontiguousarray(
            Acol[f16_js].transpose(1, 0, 2)
        ).astype(ml_dtypes.bfloat16)
        at8 = np.ascontiguousarray(
            Acol[f8_js].transpose(1, 0, 2)
        ).astype(ml_dtypes.float8_e4m3)
        ati = np.clip(
            np.round(
                np.ascontiguousarray(Acol[i8_js].transpose(1, 0, 2)) / ISCALE
            ), -127, 127,
        ).astype(np.int8)
        # bsT[(b g), nl] = bias[shard][nl, g] for both b
        bT = np.ascontiguousarray(bias[sl].T)  # [G, nl]
        bsT = np.ascontiguousarray(np.tile(bT, (B, 1))).astype(np.float16)
        in_maps.append({
            "at16": at16, "at8": at8, "ati": ati,
            "xq": xq, "kq": kq, "bsT": bsT,
        })
    return in_maps


def assemble_out(results, n=N, ncores=NCORES):
    nl = n // ncores
    parts = []
    for r in range(ncores):
        o = results[r]["outs"].reshape(B, G, nl)
        parts.append(o.transpose(0, 2, 1))  # [B, nl, G]
    return np.ascontiguousarray(np.concatenate(parts, axis=1))


def run(inputs, n=N, ncores=NCORES, trace=False, **spmd_kwargs):
    from concourse.bass_utils import run_bass_kernel_spmd

    x = np.asarray(inputs["x"], dtype=np.float32)
    A = np.asarray(inputs["A"], dtype=np.float32)
    kern = np.asarray(inputs["kernel"], dtype=np.float32)
    bias = np.asarray(inputs["bias"], dtype=np.float32)
    nc = _get_nc(n, ncores)
    in_maps = make_in_maps(x, A, kern, bias, n, ncores)
    res = run_bass_kernel_spmd(
        nc, in_maps, list(range(ncores)), trace=trace, **spmd_kwargs
    )
    out = assemble_out(res.results, n, ncores)
    return out, res


def kernel(**inputs) -> np.ndarray:
    out, _ = run(inputs)
    return out


# revision 38
# speedup vs baseline: 1.0768x; 1.0031x over previous
"""Trainium2 Bass kernel for GCNN operator:
    h   = einsum('bnf,nfg->bng', x, kernel)   # per-node feature transform
    out = einsum('nm,bmg->bng', A, h) + bias  # dense adjacency aggregation

Sharding: node dim N row-sharded across 8 cores for the A@h matmul only.
Every core redundantly computes the FULL h on its DVE (x and kernel are
small), so there is NO collective — no cross-core barrier, no skew
sensitivity. The A-shard (pre-transposed, pre-tiled on host) streams from
HBM while the TensorEngine accumulates out^T = sum_m H_m^T @ A^T_m.

The kernel is HBM-bandwidth bound on the A stream, so A is stored in
three precisions, interleaved in SPANS of 16 j-blocks:
  per span: [4 x bf16 | 5 x fp8e4 | 7 x int8 @ scale 2^-5]
bf16/fp8 tiles feed the PE directly as the moving operand (the bf16/fp8
ISA path allows 1024-row matmuls, halving PE instruction count vs fp16;
the stationary h stays fp16). int8 j-blocks are converted to bf16 by the
Activation engine (hw-measured ~1.85us/j-block; GpSimd's software path
is 10x slower and is NOT used); int * 2^-5 is exact in bf16 so the
dequant scale rides in the conversion. The fine interleave is the load-
balancing trick: the PE walks j ascending, so conversions, h chunks,
DMA arrivals and PE consumption all advance uniformly — no engine ever
waits long on a buffer-recycle semaphore (a contiguous i8 region would
stall the converter on stage-slot reuse for ~100us).

Queues: SP carries x/bias + the bf16 tiles + stores; the GpSimd queue
carries the kq chunk stream (deadline-paced for the DVE's h chunks) +
the fp8 tiles; the Activation queue carries the int8 raws, self-paced
between conversions. A few warm-up matmuls on the x tile start the PE
p-state ramp early.

Emission goes through one global event list ordered by estimated time;
the estimates only shape ORDER (writer-before-reader, slot-ring WAR
pairing, and trigger/compute interleave per engine).

Self-contained: hardcodes shapes; only imports concourse + numpy.
"""

import numpy as np

B, N, F, G = 2, 16384, 16, 16
NCORES = 8
P = 128                    # SBUF partitions
C = B * G                  # 32 fused (batch, out-feature) columns
NT = 512                   # matmul moving free-dim (ISA AP element cap)
SPAN = 16                  # j-blocks per precision span
SP16 = 5                   # bf16 j-blocks per span
SP8 = 5                    # fp8 j-blocks per span
ITILE = (3, 3)             # int8 tile sizes within a span
JJ = 8                     # j-blocks per H compute chunk
AT_BUFS = 3                # direct A-stream buffering depth
RAW_BUFS = 3               # int8 raw ring depth
STG_BUFS = 4               # converted-tile ring depth
KS_BUFS = 6                # kq chunk ring depth
ISCALE = 2.0 ** -5         # int8 quantization scale
N_PRIME = 8                # PE warm-up matmuls


def _span_plan(jn):
    """Per-j-block encoding plan. Returns (f16_js, f8_js, i8_js, spans)
    where spans is a list of per-span dicts with tile descriptors."""
    if jn >= SPAN:
        nspan = jn // SPAN
        sp16, sp8 = SP16, SP8
    else:
        nspan = 1
        sp16 = max((jn * 2 * SP16) // (2 * SPAN), 1)
        sp8 = max((jn * 2 * SP8) // (2 * SPAN), 1)
    f16_js, f8_js, i8_js, spans = [], [], [], []
    for s in range(nspan):
        base = s * (jn // nspan)
        width = jn // nspan
        n16 = sp16
        n8 = sp8
        ni = width - n16 - n8
        sp = {"f16": [], "f8": [], "i8": []}
        js16 = list(range(base, base + n16))
        js8 = list(range(base + n16, base + n16 + n8))
        jsi = list(range(base + n16 + n8, base + width))
        sp["f16"].append((len(f16_js), js16[0], n16))
        f16_js += js16
        sp["f8"].append((len(f8_js), js8[0], n8))
        f8_js += js8
        o = 0
        it = list(ITILE)
        while o < ni:
            sz = it.pop(0) if it else ni - o
            sz = min(sz, ni - o)
            sp["i8"].append((len(i8_js) + o, jsi[0] + o, sz))
            o += sz
        i8_js += jsi
        spans.append(sp)
    return f16_js, f8_js, i8_js, spans


def build_nc(n=N, ncores=NCORES, at_bufs=AT_BUFS):
    """Build the per-core Bass program (SPMD: same program on all cores)."""
    import concourse.bass as bass
    import concourse.mybir as mybir
    import concourse.tile as tile
    from concourse import bacc

    f32 = mybir.dt.float32
    f16 = mybir.dt.float16
    bf16 = mybir.dt.bfloat16
    f8 = mybir.dt.float8e4
    i8 = mybir.dt.int8

    nl = n // ncores           # local output rows per core
    jn = n // P                # contraction j-blocks over FULL n
    f16_js, f8_js, i8_js, spans = _span_plan(jn)
    n16, n8, ni = len(f16_js), len(f8_js), len(i8_js)
    ntc = min(NT, nl)          # matmul moving free-dim
    nt_n = max(nl // ntc, 1)   # acc tiles

    # H chunks of JJ j-blocks
    jj = min(JJ, jn)
    ch_sizes = [jj] * (jn // jj)
    if jn % jj:
        ch_sizes.append(jn % jj)
    nch = len(ch_sizes)
    ch_off = [0]
    for sz in ch_sizes:
        ch_off.append(ch_off[-1] + sz)
    m2ch = []
    for i, sz in enumerate(ch_sizes):
        for jl in range(sz):
            m2ch.append((i, jl))

    nc = bacc.Bacc("TRN2", target_bir_lowering=False, debug=False, num_devices=1)

    at16 = nc.dram_tensor("at16", [P, n16, nl], bf16, kind="ExternalInput")
    at8 = nc.dram_tensor("at8", [P, n8, nl], f8, kind="ExternalInput")
    ati = nc.dram_tensor("ati", [P, ni, nl], i8, kind="ExternalInput")
    xq = nc.dram_tensor("xq", [P, jn * B * F], f16, kind="ExternalInput")
    kq = nc.dram_tensor("kq", [P, jn * G * F], f16, kind="ExternalInput")
    bsT = nc.dram_tensor("bsT", [C, nl], f16, kind="ExternalInput")
    outs = nc.dram_tensor("outs", [C, nl], f32, kind="ExternalOutput")

    kq_r = kq.ap().rearrange("p (j g f) -> p j g f", g=G, f=F)

    # ---- timing estimates (µs), used only for emission ordering ----
    H_DT = 7.3                 # µs per DVE h chunk (2 bcast mults + 2 reduces)
    CONV_US = 1.9 * nl / 2048  # µs per converted j-block on ACT
    WIRE = 2.6                 # aggregate stream µs per MB
    mbytes = lambda nblk, bpe: P * nl * nblk * bpe / 1e6

    events = []

    def ev(t, fn):
        events.append((t, len(events), fn))

    with tile.TileContext(nc) as tc:
        with (
            tc.tile_pool(name="const", bufs=1) as const,
            tc.tile_pool(name="work", bufs=2) as work,
            tc.tile_pool(name="ksp", bufs=KS_BUFS) as ksp,
            tc.tile_pool(name="atp", bufs=at_bufs) as atp,
            tc.tile_pool(name="rawp", bufs=RAW_BUFS) as rawp,
            tc.tile_pool(name="stgp", bufs=STG_BUFS) as stgp,
            tc.tile_pool(name="pacc", bufs=1, space="PSUM") as pacc,
        ):
            xs = const.tile([P, jn, B, F], f16)
            biasT = const.tile([C, nl], f16)
            ks_tiles = [None] * nch
            hqs = [
                const.tile([P, ch_sizes[q], C], f16, tag=f"hq{q}",
                           name=f"hq{q}")
                for q in range(nch)
            ]
            outT = work.tile([C, nl], f32, tag="outT")
            acc = [
                pacc.tile([C, ntc], f32, tag=f"acc{t}", name=f"acc{t}")
                for t in range(nt_n)
            ]
            pw = min(512, jn * B * F)
            pprime = pacc.tile([C, pw], f32, tag="pprime")
            xs_flat = xs.rearrange("p a b c -> p (a b c)")

            # -- prologue: xs first on the sync queue while ks0 rides the
            # gpsimd queue in parallel — both land ~11us, h starts ~13us --
            ev(0.0, lambda: nc.sync.dma_start(
                out=xs[:, :, :, :],
                in_=xq.ap().rearrange("p (j b f) -> p j b f", b=B, f=F),
            ))
            ev(0.01, lambda: nc.sync.dma_start(out=biasT[:, :], in_=bsT.ap()))

            # -- PE warm-up: p-state ramp starts once x lands --
            def prime():
                for _ in range(N_PRIME):
                    nc.tensor.matmul(
                        pprime[:, :], xs_flat[:, 0:C], xs_flat[:, 0:pw],
                        start=True, stop=True,
                    )
            ev(0.02, prime)

            # -- kq chunk ring: the gpsimd queue's TOP priority (nothing
            # big rides ahead of a due ks chunk — a late ks chunk stalls
            # the DVE h chain and with it the whole kernel) --
            for q in range(nch):
                def load_ks(q=q):
                    kt = ksp.tile([P, jj, G, F], f16, tag="ks",
                                  name=f"ks{q}")
                    nc.gpsimd.dma_start(
                        out=kt[:, : ch_sizes[q], :, :],
                        in_=kq_r[:, ch_off[q] : ch_off[q + 1]],
                    )
                    ks_tiles[q] = kt
                if q < 3:
                    t = 0.02 + 0.01 * q
                else:
                    t = H_DT * (q - 2)
                ev(t, load_ks)

            # -- h chunks on the DVE (per-b broadcast mult + reduce; the
            # single-broadcast mult hits the DVE's packed 2x mode; NOTE:
            # GpSimd must NOT help here — it shares an SBUF port with the
            # DVE under an exclusive lock and serializes both) --
            for q in range(nch):
                sz = ch_sizes[q]
                def hstep(q=q, sz=sz):
                    with nc.allow_low_precision(
                        reason="h accum over F=16 in fp16"
                    ):
                        prod = work.tile([P, jj, B, G, F], f16, tag="prod")
                        for b in range(B):
                            nc.vector.tensor_tensor(
                                prod[:, :sz, b, :, :],
                                xs[
                                    :, ch_off[q] : ch_off[q + 1], b, None, :
                                ].to_broadcast([P, sz, G, F]),
                                ks_tiles[q][:, :sz, :, :],
                                mybir.AluOpType.mult,
                            )
                        nc.vector.tensor_reduce(
                            hqs[q][:, :, :].rearrange(
                                "p j (b g) -> p j b g", b=B
                            ),
                            prod[:, :sz, :, :, :],
                            axis=mybir.AxisListType.X,
                            op=mybir.AluOpType.add,
                        )
                ev(1.0 + H_DT * q, hstep)

            # -- per-tile emitters --
            at_tiles = {}
            raw_tiles = {}
            stg_tiles = {}

            def emit_direct(key, seg, so, sz, eng_i):
                def trig():
                    dt_t = bf16 if seg == 0 else f8
                    src = at16 if seg == 0 else at8
                    at_t = atp.tile([P, sz, nl], dt_t, tag="at_t",
                                    name="at_t")
                    eng = [nc.sync, nc.scalar, nc.gpsimd][eng_i]
                    eng.dma_start(
                        out=at_t[:, :, :], in_=src.ap()[:, so : so + sz, :]
                    )
                    at_tiles[key] = at_t
                return trig

            def emit_rawload(key, so, sz, eng_i):
                def rawload():
                    rt = rawp.tile([P, max(ITILE), nl], i8, tag="rawt",
                                   name=f"raw{key}")
                    eng = [nc.sync, nc.scalar, nc.gpsimd][eng_i]
                    eng.dma_start(
                        out=rt[:, :sz, :], in_=ati.ap()[:, so : so + sz, :]
                    )
                    raw_tiles[key] = rt
                return rawload

            def emit_conv(key, sz):
                def conv():
                    rt = raw_tiles[key]
                    st = stgp.tile([P, max(ITILE), nl], bf16, tag="stgt",
                                   name=f"stg{key}")
                    with nc.allow_low_precision(
                        reason="int8*2^-5 dequant is exact in bf16"
                    ):
                        nc.scalar.activation(
                            st[:, :sz, :], rt[:, :sz, :],
                            mybir.ActivationFunctionType.Copy, scale=ISCALE,
                        )
                    stg_tiles[key] = st
                return conv

            # -- build the PE consumption sequence: span by span, j
            # ascending: f16 tile, f8 tile, then the span's i8 tiles --
            seq = []  # (kind, key, seg_off, abs_j0, sz)
            for sp in spans:
                for so, j0, sz in sp["f16"]:
                    seq.append(("d0", len(seq), so, j0, sz))
                for so, j0, sz in sp["f8"]:
                    seq.append(("d1", len(seq), so, j0, sz))
                for so, j0, sz in sp["i8"]:
                    seq.append(("s", len(seq), so, j0, sz))

            # -- sequential estimate pass: every writer lands before its
            # readers; each slot ring's writer k+bufs after reads of k --
            h_ready = [1.0 + H_DT * (q + 1) for q in range(nch)]
            mm_est = []
            wire_c = 0.5
            conv_busy = 6.0
            d_hist = []  # seq indices of direct tiles (atp ring order)
            s_hist = []  # seq indices of staged tiles (stg/raw rings)
            conv_ends = []  # conv completion est per staged tile
            for si, (kind, key, so, j0, sz) in enumerate(seq):
                if kind in ("d0", "d1"):
                    seg = 0 if kind == "d0" else 1
                    t = wire_c
                    if len(d_hist) >= at_bufs:
                        t = max(t, mm_est[d_hist[-at_bufs]] + 0.06)
                    # f16 tiles alternate SP/ACT queues; f8 rides SP (the
                    # gpsimd queue is reserved for ks + late raws)
                    eng_i = (len(d_hist) % 2) if seg == 0 else 0
                    ev(t, emit_direct(key, seg, so, sz, eng_i))
                    wire_c = t + mbytes(sz, 2 if seg == 0 else 1) * WIRE
                    avail = t + 0.1
                    d_hist.append(si)
                else:
                    raw_t = max(conv_busy - 5.0, 0.06)
                    conv_t = conv_busy
                    if len(s_hist) >= RAW_BUFS:
                        # raw slot reader is the RAW_BUFS-back conversion
                        raw_t = max(raw_t, conv_ends[-RAW_BUFS] + 0.02)
                    if len(s_hist) >= STG_BUFS:
                        # stage slot reader is the STG_BUFS-back matmul
                        conv_t = max(
                            conv_t, mm_est[s_hist[-STG_BUFS]] + 0.06
                        )
                    # early raws ride the ACT queue, late ones gpsimd
                    # (free once the ks stream drains)
                    reng = 1 if len(s_hist) < 12 else 2
                    ev(raw_t, emit_rawload(key, so, sz, reng))
                    ev(conv_t, emit_conv(key, sz))
                    conv_busy = conv_t + CONV_US * sz + 0.3
                    conv_ends.append(conv_busy)
                    avail = conv_busy + 0.05
                    s_hist.append(si)
                q_last = m2ch[j0 + sz - 1][0]
                t = max(mm_est[-1] + 0.4 if mm_est else 0.4,
                        avail, h_ready[q_last] + 0.05)
                mm_est.append(t)

            def mmtile(si, final):
                kind, key, so, j0, sz = seq[si]
                def go(kind=kind, key=key, j0=j0, sz=sz, final=final):
                    tile_ap = at_tiles[key] if kind != "s" else stg_tiles[key]
                    if not final:
                        for kk in range(sz):
                            m = j0 + kk
                            q, jl = m2ch[m]
                            for t in range(nt_n):
                                nc.tensor.matmul(
                                    acc[t][:, :], hqs[q][:, jl, :],
                                    tile_ap[:, kk, t * ntc : (t + 1) * ntc],
                                    start=(m == 0), stop=False,
                                )
                    else:
                        # Final tile: t-outer so each acc closes in turn;
                        # drain to SBUF with fused bias add and store while
                        # the PE still works the later t slices.
                        for t in range(nt_n):
                            for kk in range(sz):
                                m = j0 + kk
                                q, jl = m2ch[m]
                                nc.tensor.matmul(
                                    acc[t][:, :], hqs[q][:, jl, :],
                                    tile_ap[:, kk, t * ntc : (t + 1) * ntc],
                                    start=(m == 0), stop=(kk == sz - 1),
                                )
                            nc.vector.tensor_add(
                                outT[:, t * ntc : (t + 1) * ntc],
                                acc[t][:, :],
                                biasT[:, t * ntc : (t + 1) * ntc],
                            )
                            eng2 = nc.scalar if t % 2 else nc.sync
                            eng2.dma_start(
                                out=outs.ap()[:, t * ntc : (t + 1) * ntc],
                                in_=outT[:, t * ntc : (t + 1) * ntc],
                            )
                return go

            for si in range(len(seq)):
                ev(mm_est[si], mmtile(si, si == len(seq) - 1))

            # -- emit everything in estimated-time order --
            events.sort(key=lambda e: (e[0], e[1]))
            for _, _, fn in events:
                fn()

    nc.compile()
    return nc


_NC_CACHE = {}


def _get_nc(n=N, ncores=NCORES):
    key = (n, ncores)
    if key not in _NC_CACHE:
        _NC_CACHE[key] = build_nc(n, ncores)
    return _NC_CACHE[key]


def make_in_maps(x, A, kern, bias, n=N, ncores=NCORES):
    import ml_dtypes

    nl = n // ncores
    jn = n // P
    f16_js, f8_js, i8_js, _ = _span_plan(jn)

    # Shared across cores: x and kernel in [p-major] DVE-friendly layouts.
    # xq[p, j, b, f] = x[b, j*P+p, f];  kq[p, j, g, f] = kern[j*P+p, f, g]
    x16 = x.astype(np.float16).transpose(1, 0, 2).reshape(jn, P, B, F)
    xq = np.ascontiguousarray(x16.transpose(1, 0, 2, 3)).reshape(P, jn * B * F)
    k16 = kern.astype(np.float16).transpose(0, 2, 1).reshape(jn, P, G, F)
    kq = np.ascontiguousarray(k16.transpose(1, 0, 2, 3)).reshape(P, jn * G * F)

    in_maps = []
    for r in range(ncores):
        sl = slice(r * nl, (r + 1) * nl)
        # Acol[j, p, :] = A^T[j*P + p, r-shard] (fp32, cast per region)
        Acol = A[sl, :].T.reshape(jn, P, nl)
        at16 = np.ascontiguousarray(
            Acol[f16_js].transpose(1, 0, 2)
        ).astype(ml_dtypes.bfloat16)
        at8 = np.ascontiguousarray(
            Acol[f8_js].transpose(1, 0, 2)
        ).astype(ml_dtypes.float8_e4m3)
        ati = np.clip(
            np.round(
                np.ascontiguousarray(Acol[i8_js].transpose(1, 0, 2)) / ISCALE
            ), -127, 127,
        ).astype(np.int8)
        # bsT[(b g), nl] = bias[shard][nl, g] for both b
        bT = np.ascontiguousarray(bias[sl].T)  # [G, nl]
        bsT = np.ascontiguousarray(np.tile(bT, (B, 1))).astype(np.float16)
        in_maps.append({
            "at16": at16, "at8": at8, "ati": ati,
            "xq": xq, "kq": kq, "bsT": bsT,
        })
    return in_maps


def assemble_out(results, n=N, ncores=NCORES):
    nl = n // ncores
    parts = []
    for r in range(ncores):
        o = results[r]["outs"].reshape(B, G, nl)
        parts.append(o.transpose(0, 2, 1))  # [B, nl, G]
    return np.ascontiguousarray(np.concatenate(parts, axis=1))


def run(inputs, n=N, ncores=NCORES, trace=False, **spmd_kwargs):
    from concourse.bass_utils import run_bass_kernel_spmd

    x = np.asarray(inputs["x"], dtype=np.float32)
    A = np.asarray(inputs["A"], dtype=np.float32)
    kern = np.asarray(inputs["kernel"], dtype=np.float32)
    bias = np.asarray(inputs["bias"], dtype=np.float32)
    nc = _get_nc(n, ncores)
    in_maps = make_in_maps(x, A, kern, bias, n, ncores)
    res = run_bass_kernel_spmd(
        nc, in_maps, list(range(ncores)), trace=trace, **spmd_kwargs
    )
    out = assemble_out(res.results, n, ncores)
    return out, res


def kernel(**inputs) -> np.ndarray:
    out, _ = run(inputs)
    return out
